# revision 33
# baseline (speedup 1.0000x reference)
"""Multi-head attention (B=2, N=2048, C=768, H=12, DH=64) on 8 Trainium2 cores.

Sharding: data-parallel on batch (cores 0-3 -> b=0, cores 4-7 -> b=1),
tensor-parallel on heads within each group (3 heads/core: Wq/Wk/Wv column
slices, Wp row slices).  Each core emits its partial projection output
[N, C]; the host sums the 4 partials per batch and adds bp (cheaper than a
device collective at this size).

Per-core dataflow (feature-major, transpose-free, fp16 operands / fp32 psum):
  - host supplies xT = x[b].T  [C, N] in fp16; h2's q and k weight columns
    are host-packed into one [C,128] tensor so all qk-proj matmuls are M=128
  - PE warmup: dummy matmuls on a memset tile run during the initial DMA
    wait so the HAM clock-gate reaches K=8/8 before real work starts; a
    tiny Exp at t=0 preloads the ACT table set (~2.7us off the stream);
    more dummies bridge the phase-1 -> stream pool barrier (any PE gap
    >~2.5us re-throttles the clock gate for 7-34us)
  - qT,kT [64, N] per head = W.T @ xT; biases via the ACT Identity bias
    port (per-partition column) instead of K=1 matmuls; each head's 64
    dims then duplicated onto both PE-row halves so score matmuls pair
    even/odd kj tiles on disjoint PE row halves (co-execute)
  - h2's psum drains through ACT only (nt-outer, pipelined per-nt), so
    the pool-close barrier that gates the v/score psum pools never waits
    on the DVE queue (which carries the big h0/h1 dup copies)
  - v [N, 192] token-major from xT as lhsT, with a ones column per head;
    bv added by the DVE psum->sbuf copy against a gpsimd-broadcast bias
    tile, not a K=1 matmul; a few v tiles are emitted BEFORE the first
    score groups so the PE stays busy across the transition
  - phase 3 is one continuous stream over (qq, head, kj): 192 score tiles
    STt [kj,qi] grouped 3 per [128,1536] psum tile; ONE exp ACT op per
    group; yT accumulation consumes ET LAG groups behind so transient PE
    detours (proj tiles, normalize) never starve the ACT engine
  - yT_aug[65, qi] = [v_h | 1].T @ ET accumulated over kj; row 64 = denom
  - normalize: denom -> sbuf copy, reciprocal_approx_fast, gpsimd
    partition_broadcast, fused multiply deferred two groups
  - out[qi, C] partial = yT (stationary) @ Wp rows: one single-group
    detour per row-tile (A,A then B,B so each stationary loads once; the
    trailing partB co-executes with the next group's odd-kj score MM);
    the final four tiles pipeline chunked-normalize -> partB pair ->
    casts (split DVE/ACT) -> two half-DMAs per tile
"""

import math

import numpy as np

import concourse.bacc as bacc
import concourse.bass as bass
import concourse.mybir as mybir
import concourse.tile as tile
from concourse import bass_utils

B, N, C, H, DH = 2, 2048, 768, 12, 64
NCORES = 8
CPG = 4                  # cores per batch group
HPC = H // CPG           # heads per core = 3
MYC = HPC * DH           # per-core feature width = 192
KC = C // 128            # contraction chunks = 6
NTT = N // 128           # token tiles = 16
QB = 512                 # qi block (psum bank width, fp32)
LAG = 7                  # ET ring depth: yt consumption trails ACT by LAG
NWARM = 14               # dummy warmup matmuls during the initial DMA wait
                         # (~4.5us: engines unblock ~7.5us after t0, the
                         # first xT chunk lands ~11.7us; more would push the
                         # real matmuls back since the PE queue is FIFO)
F32 = mybir.dt.float32
I16 = mybir.dt.int16
MMDT = mybir.dt.float16  # matmul operand dtype
AF = mybir.ActivationFunctionType
OP = mybir.AluOpType

EXP_SHIFT = -3.0         # exp(s + EXP_SHIFT); cancels between num and denom
# fp16-domain Schraudolph fast exp: bits16(exp(s-3)) ~ round(A16*s + B16).
# Scores are in [-6.1, +6.3] (fixed input seed), so bits stay in
# [~1870, ~20200] -- no int16 overflow, no sign-flip underflow.
F16A = 1024.0 / math.log(2.0)                      # 2^10 * log2(e)
F16B = 15360.0 - 44.7 + EXP_SHIFT * F16A
# groups whose exp runs on DVE (int16 Schraudolph) instead of ACT.
# Scattered offload groups leave a +-3% sawtooth on just THOSE token
# blocks' weights; rows whose attention concentrates there see the full
# error (sim: 1.98e-2, at the gate).  Offload is only safe per whole
# (qq,h) row -- the common-mode error then cancels in the softmax ratio.
DVE_EXP_GROUPS = frozenset()


def _bcast_parts(ap, nparts):
    """Partition-stride-0 broadcast view of a [1, F] AP (DMA source only)."""
    return bass.AP(tensor=ap.tensor, offset=ap.offset,
                   ap=[[0, nparts]] + [list(d) for d in ap.ap[1:]])


def _emit(nc, tc, pools, aps):
    xT, wq, wk, wqk2, wv, wp, bq, bk, bqk2, bv, out = (
        aps["xT"], aps["wq"], aps["wk"], aps["wqk2"], aps["wv"], aps["wp"],
        aps["bq"], aps["bk"], aps["bqk2"], aps["bv"], aps["out"],
    )
    persist = pools["persist"]
    et_pool = pools["et"]
    small = pools["small"]
    ostage = pools["ostage"]

    # ---- persistent SBUF tensors ----
    xT_sb = persist.tile([128, KC * N], MMDT, tag="xT_sb")
    wq_sb = persist.tile([128, KC * 128], MMDT, tag="wq_sb")
    wk_sb = persist.tile([128, KC * 128], MMDT, tag="wk_sb")
    wqk2_sb = persist.tile([128, KC * 128], MMDT, tag="wqk2_sb")
    wv_sb = persist.tile([128, KC * MYC], MMDT, tag="wv_sb")
    wpA = persist.tile([128, C], MMDT, tag="wpA")
    # wpB rows 64:128 duplicate rows 0:64 (same DRAM source, two DMAs) so
    # the final partB matmuls can run on either PE row half
    wpB = persist.tile([128, C], MMDT, tag="wpB")
    bqc = persist.tile([128, 1], F32, tag="bqc")
    bkc = persist.tile([128, 1], F32, tag="bkc")
    bqk2c = persist.tile([128, 1], F32, tag="bqk2c")
    bv_row = persist.tile([1, MYC], F32, tag="bv_row")
    bv_bc = persist.tile([128, MYC], F32, tag="bv_bc")
    shift_col = persist.tile([128, 1], F32, tag="shift_col")
    warm_x = persist.tile([128, QB], MMDT, tag="warm_x")
    tiny = persist.tile([1, 8], F32, tag="tiny")
    tiny2 = persist.tile([1, 8], F32, tag="tiny2")
    # compact projections (h0 on parts 0:64, h1 on 64:128; h2 separate)
    qTA = persist.tile([128, N], MMDT, tag="qTA")
    kTA = persist.tile([128, N], MMDT, tag="kTA")
    # partition-duplicated k/q halves for even/odd kj pair packing:
    # qTDx packs h0's dup (upper half) + h1's dup (lower half); h0-even
    # and h1-odd read qTA/kTA directly; h2 fully duplicated in qTD2
    qTDx = persist.tile([128, N], MMDT, tag="qTDx")
    kTDx = persist.tile([128, N], MMDT, tag="kTDx")
    qTD2 = persist.tile([128, N], MMDT, tag="qTD2")
    kTD2 = persist.tile([128, N], MMDT, tag="kTD2")
    v_sb = persist.tile([128, NTT * HPC * 65], MMDT, tag="v_sb")
    yTA = persist.tile([128, N], MMDT, tag="yTA")
    # yTB rows 0:64 = h2's yT; rows 64:128 = a dup of the last block so the
    # final partB matmuls co-execute pairwise on disjoint PE row halves
    yTB = persist.tile([128, N], MMDT, tag="yTB")

    # warmup operand + ACT exp-table preload: both ready ~instantly so the
    # PE and ACT start before the first input DMA lands
    nc.vector.memset(warm_x, 0.02)
    nc.vector.memset(tiny, 0.0)
    nc.scalar.activation(tiny2, tiny, AF.Exp)

    # ---- input DMAs: qk-proj operands first so phase 1 starts ASAP ----
    # only the three tensors the q/k passes consume go in the hot loop;
    # wqk2 (needed ~29us in) would steal 25% of the early HBM bandwidth
    # and stall the DMA-paced q/k matmul interleave
    for kc in range(KC):
        nc.sync.dma_start(out=xT_sb[:, kc * N:(kc + 1) * N],
                          in_=xT[kc * 128:(kc + 1) * 128, :])
        nc.sync.dma_start(out=wq_sb[:, kc * 128:(kc + 1) * 128],
                          in_=wq[kc * 128:(kc + 1) * 128, :])
        nc.sync.dma_start(out=wk_sb[:, kc * 128:(kc + 1) * 128],
                          in_=wk[kc * 128:(kc + 1) * 128, :])
    nc.sync.dma_start(out=bqc, in_=bq)
    nc.sync.dma_start(out=bkc, in_=bk)
    nc.sync.dma_start(out=bqk2c, in_=bqk2)
    for kc in range(KC):
        nc.sync.dma_start(out=wqk2_sb[:, kc * 128:(kc + 1) * 128],
                          in_=wqk2[kc * 128:(kc + 1) * 128, :])
    for kc in range(KC):
        nc.sync.dma_start(out=wv_sb[:, kc * MYC:(kc + 1) * MYC],
                          in_=wv[kc * 128:(kc + 1) * 128, :])
    nc.sync.dma_start(out=bv_row, in_=bv)
    nc.sync.dma_start(out=wpA, in_=wp[0:128, :])
    nc.sync.dma_start(out=wpB[0:64, :], in_=wp[128:MYC, :])
    nc.sync.dma_start(out=wpB[64:128, :], in_=wp[128:MYC, :])
    nc.gpsimd.partition_broadcast(bv_bc, bv_row)
    nc.vector.memset(shift_col, EXP_SHIFT)
    # pre-fill v_sb with 1.0: the per-head ones-columns (denominator rows
    # of the yt matmuls) then need no per-tile copies; the 64-wide value
    # copies overwrite their slices
    nc.vector.memset(v_sb, 1.0)

    # ---- phase 1: q/k/h2-combined projections (M=128 passes) ----
    with tc.tile_pool(name="ps_qk", bufs=2, space="PSUM") as ps_qk:
        pssQ = [ps_qk.tile([128, QB], F32, tag="ps_qkA", bufs=4,
                           name=f"ps_q{_i}") for _i in range(N // QB)]
        pssK = [ps_qk.tile([128, QB], F32, tag="ps_qkB", bufs=4,
                           name=f"ps_k{_i}") for _i in range(N // QB)]
        # HAM warmup: dummy matmuls on the memset tile keep the PE busy
        # (and the clock gate at 8/8) while the first xT/w DMAs land; the
        # real kc=0 matmul re-opens the bank with start=True
        for i in range(NWARM):
            nc.tensor.matmul(pssQ[0], warm_x[:, 0:128], warm_x,
                             start=(i == 0), stop=(i == NWARM - 1))
        # q and k interleaved per kc chunk so matmul consumption stays
        # behind the xT DMA supply
        for kc in range(KC):
            for nt in range(N // QB):
                nc.tensor.matmul(
                    pssQ[nt], wq_sb[:, kc * 128:(kc + 1) * 128],
                    xT_sb[:, kc * N + nt * QB: kc * N + nt * QB + QB],
                    start=(kc == 0), stop=(kc == KC - 1))
            for nt in range(N // QB):
                nc.tensor.matmul(
                    pssK[nt], wk_sb[:, kc * 128:(kc + 1) * 128],
                    xT_sb[:, kc * N + nt * QB: kc * N + nt * QB + QB],
                    start=(kc == 0), stop=(kc == KC - 1))
        # psum->sbuf move on the (pre-stream idle) ACT engine with the
        # bias folded into the activation's per-partition bias port
        for nt in range(N // QB):
            nc.scalar.activation(qTA[:, nt * QB:(nt + 1) * QB], pssQ[nt],
                                 AF.Identity, bias=bqc)
            nc.scalar.activation(kTA[:, nt * QB:(nt + 1) * QB], pssK[nt],
                                 AF.Identity, bias=bkc)
        # h0/h1 duplicated halves can be built as soon as q/k biases land
        # (DVE; gpsimd's tensor_copy is ~10x slower).  The [64:128] dups
        # feed the very first score groups (h0 odd-kj), so they go first.
        nc.vector.tensor_copy(out=qTDx[64:128, :], in_=qTA[0:64, :])
        nc.vector.tensor_copy(out=kTDx[64:128, :], in_=kTA[0:64, :])
        nc.vector.tensor_copy(out=qTDx[0:64, :], in_=qTA[64:128, :])
        nc.vector.tensor_copy(out=kTDx[0:64, :], in_=kTA[64:128, :])
        # combined h2 pass: psum rows 0:64 = q-h2, rows 64:128 = k-h2
        pss2 = [ps_qk.tile([128, QB], F32, tag="ps_qkA", bufs=4,
                           name=f"ps_2{_i}") for _i in range(N // QB)]
        # nt-OUTER so each pss2 tile finishes early and its drain chain
        # (ACT copy + DVE cast + dups, ~1.3us) pipelines under the next
        # nt's matmuls; kc-outer would serialize the whole ~5us drain
        # after the last h2 matmul, and everything downstream (the v/score
        # psum pools) waits on that drain via the pool-close barrier
        for nt in range(N // QB):
            for kc in range(KC):
                nc.tensor.matmul(
                    pss2[nt], wqk2_sb[:, kc * 128:(kc + 1) * 128],
                    xT_sb[:, kc * N + nt * QB: kc * N + nt * QB + QB],
                    start=(kc == 0), stop=(kc == KC - 1))
            sl = slice(nt * QB, (nt + 1) * QB)
            # BOTH h2 psum copies on ACT (idle pre-stream): then only ACT
            # reads pss2, so the pool-close barrier doesn't wait on the
            # DVE queue (which carries the big h0/h1 dups); the dup chunks
            # (SBUF->SBUF, not pss2 readers) go on DVE and pipeline per-nt
            nc.scalar.activation(kTD2[64:128, sl], pss2[nt][64:128, :],
                                 AF.Identity, bias=bqk2c[64:128, :])
            nc.scalar.activation(qTD2[0:64, sl], pss2[nt][0:64, :],
                                 AF.Identity, bias=bqk2c[0:64, :])
            nc.vector.tensor_copy(out=qTD2[64:128, sl], in_=qTD2[0:64, sl])
            nc.vector.tensor_copy(out=kTD2[0:64, sl], in_=kTD2[64:128, sl])
        # PE filler: the pool-close barrier waits ~2us for the last pss2
        # drain; dummy matmuls (no deps) keep the PE busy so the HAM
        # activity monitor never sees an idle window and re-throttles
        pfill = ps_qk.tile([128, QB], F32, tag="ps_qkA", bufs=4,
                           name="pfill")
        for i in range(14):
            nc.tensor.matmul(pfill, warm_x[:, 0:128], warm_x,
                             start=(i == 0), stop=(i == 13))

    # ---- phases 2+3: v projection + score stream share the PSUM pools ----
    def vh_ap(kj, h):
        base = (kj * HPC + h) * 65
        return v_sb[:, base:base + 65]

    # normalize phase 1: denom row -> sbuf, fast reciprocal, then a
    # gpsimd partition_broadcast (all-SBUF, so legal on Pool).  The fused
    # multiply (phase 2) is DEFERRED two groups so its wait never
    # head-of-line-blocks the DVE queue.
    def norm_start(yt, h, qq):
        den = small.tile([1, QB], F32, tag="den")
        nc.vector.tensor_copy(out=den, in_=yt[64:65, :])
        rec = small.tile([1, QB], F32, tag="rec")
        nc.vector.reciprocal_approx_fast(rec, den)
        bc = small.tile([64, QB], F32, tag="bc_sb")
        nc.gpsimd.partition_broadcast(bc, rec)
        return (yt, bc, h, qq)

    def norm_finish(state):
        yt, bc, h, qq = state
        q0 = qq * QB
        ydst = yTA[0:64, :] if h == 0 else (
            yTA[64:128, :] if h == 1 else yTB[0:64, :])
        nc.vector.scalar_tensor_tensor(
            out=ydst[:, q0:q0 + QB], in0=yt[0:64, :], scalar=1.0, in1=bc,
            op0=OP.mult, op1=OP.mult,
        )

    def proj_full(ps_st, qt):
        # one output row-tile per detour: A,A then B,B (each stationary
        # loaded once), casts, DMA.  A single ~1us detour holds the st
        # slot only ~1.5 groups (vs 2.5 when split across two groups), so
        # the score stream's slot handoff doesn't stall the PE; the
        # trailing partB matmul co-executes with the next group's odd-kj
        # score matmul (disjoint PE row halves)
        stt = ps_st.tile([128, 3 * QB], F32, tag="st", name=f"pj{qt}")
        ob = ostage.tile([128, C], MMDT, tag="ob", name=f"ob{qt}")
        for nb in range(2):
            nc.tensor.matmul(stt[:, nb * QB: nb * QB + 384],
                             yTA[:, qt * 128:(qt + 1) * 128],
                             wpA[:, nb * 384:(nb + 1) * 384],
                             start=True, stop=False)
        for nb in range(2):
            nc.tensor.matmul(stt[:, nb * QB: nb * QB + 384],
                             yTB[0:64, qt * 128:(qt + 1) * 128],
                             wpB[0:64, nb * 384:(nb + 1) * 384],
                             start=False, stop=True)
        for nb in range(2):
            nc.vector.tensor_copy(
                out=ob[:, nb * 384:(nb + 1) * 384],
                in_=stt[:, nb * QB: nb * QB + 384])
        nc.sync.dma_start(out=out[qt * 128:(qt + 1) * 128, :], in_=ob)

    stream = [(qq, h, kj)
              for qq in range(4) for h in range(HPC) for kj in range(NTT)]
    NG = len(stream) // 3  # 64 groups of 3 score tiles

    # proj for block qq interleaved into block qq+1's stream, one nb-half
    # per group; slot allocations stay 2 groups apart (parity-preserving);
    # keyed by CONSUMED group
    proj_at = {}
    # even goff: fires at an odd consumed-group, so the next EMITTED
    # group (cg+LAG+1, LAG=7) starts with an odd-kj score matmul on PE
    # rows 64:128 -- the trailing partB matmul (rows 0:64) co-executes
    for qq in range(3):
        for j, goff in enumerate((8, 10, 12, 14)):
            qt = qq * 4 + j
            proj_at.setdefault((qq + 1) * 16 + goff, []).append(qt)

    def st_srcs(h, kj):
        if kj % 2 == 0:      # PE rows 0:64
            kt, qt_ = ((kTA, qTA), (kTDx, qTDx), (kTD2, qTD2))[h]
            lo = 0
        else:                # PE rows 64:128
            kt, qt_ = ((kTDx, qTDx), (kTA, qTA), (kTD2, qTD2))[h]
            lo = 64
        return kt, qt_, lo

    def emit_st_group(ps_st, g):
        entries = [stream[3 * g + j] for j in range(3)]
        st = ps_st.tile([128, 3 * QB], F32, tag="st", name=f"st{g}")
        for j, (qq, h, kj) in enumerate(entries):
            kt, qt_, lo = st_srcs(h, kj)
            nc.tensor.matmul(
                st[:, j * QB:(j + 1) * QB],
                kt[lo:lo + 64, kj * 128:(kj + 1) * 128],
                qt_[lo:lo + 64, qq * QB:(qq + 1) * QB],
                start=True, stop=True,
            )
        if g in DVE_EXP_GROUPS:
            # fp16-domain Schraudolph: one DVE op producing the fp16 BIT
            # pattern as int16; the ring stores the tile + a bitcast flag
            eti = et_pool.tile([128, 3 * QB], I16, tag="et", name=f"et{g}")
            nc.vector.tensor_scalar(out=eti, in0=st, scalar1=F16A,
                                    scalar2=F16B, op0=OP.mult, op1=OP.add)
            return (eti, True, entries)
        et = et_pool.tile([128, 3 * QB], MMDT, tag="et", name=f"et{g}")
        nc.scalar.activation(et, st, AF.Exp, bias=shift_col[:, :])
        return (et, False, entries)

    with tc.tile_pool(name="ps_st", bufs=2, space="PSUM") as ps_st, \
         tc.tile_pool(name="ps_yt", bufs=2, space="PSUM") as ps_yt:
        ring = []

        def emit_v(nt):
            ps = ps_yt.tile([128, MYC], F32, tag="yt", name=f"ps_v{nt}")
            for kc in range(KC):
                nc.tensor.matmul(
                    ps,
                    xT_sb[:, kc * N + nt * 128: kc * N + nt * 128 + 128],
                    wv_sb[:, kc * MYC:(kc + 1) * MYC],
                    start=(kc == 0), stop=(kc == KC - 1),
                )
            for h in range(HPC):
                base = (nt * HPC + h) * 65
                nc.vector.tensor_tensor(
                    out=v_sb[:, base:base + 64],
                    in0=ps[:, h * 64:(h + 1) * 64],
                    in1=bv_bc[:, h * 64:(h + 1) * 64], op=OP.add)

        # v tiles FIRST: they depend only on long-resident xT/wv and their
        # psum pool's banks drained early, so the PE stays busy during the
        # ~2us h2-psum drain that gates the first score groups (a PE gap
        # here costs a HAM re-throttle, worth tens of us)
        for nt in range(4):
            emit_v(nt)
        ring.append(emit_st_group(ps_st, 0))
        ring.append(emit_st_group(ps_st, 1))
        for k in range(2, LAG):
            s = 4 + (NTT - 4) * (k - 2) // (LAG - 2)
            e = 4 + (NTT - 4) * (k - 1) // (LAG - 2)
            for nt in range(s, e):
                emit_v(nt)
            ring.append(emit_st_group(ps_st, k))

        # ---- phase 3 main loop ----
        yt_cur = {}
        pending = []   # (due consumed-group, norm state)
        pjAB = []
        consumed = 0
        gi = LAG
        while consumed < NG:
            if gi < NG:
                ring.append(emit_st_group(ps_st, gi))
            gi += 1
            ncons = min(1 if gi <= NG else 2, len(ring))
            batch, ring[:] = ring[:ncons], ring[ncons:]
            for pet, is_i16, pentries in batch:
                cg = consumed
                consumed += 1
                for j, (qq, h, kj) in enumerate(pentries):
                    if kj == 0:
                        yt_cur[(qq, h)] = ps_yt.tile(
                            [65, QB], F32, tag="yt", name=f"yt{qq}_{h}")
                    src = pet[:, j * QB:(j + 1) * QB]
                    if is_i16:
                        src = src.bitcast(MMDT)
                    nc.tensor.matmul(yt_cur[(qq, h)], vh_ap(kj, h), src,
                                     start=(kj == 0), stop=(kj == NTT - 1))
                    if kj == NTT - 1:
                        pending.append(
                            (cg + 2, norm_start(yt_cur.pop((qq, h)), h, qq)))
                while pending and pending[0][0] <= cg:
                    norm_finish(pending.pop(0)[1])
                for qt in proj_at.get(cg + 1, []):
                    proj_full(ps_st, qt)
        # final block's h0+h1 proj contribution emitted after every other
        # st-pool user (the partA tiles hold both slots until partB):
        # 8 bank-aligned po regions (3+3 in the st slots, 2 from yt pool)
        stA = ps_st.tile([128, 3 * QB], F32, tag="st", name="pjtA")
        stB = ps_st.tile([128, 3 * QB], F32, tag="st", name="pjtB")
        poY = [ps_yt.tile([128, 384], F32, tag="yt", name=f"poY{_i}")
               for _i in range(2)]
        # po regions are spread so consecutive qt's regions live on
        # DIFFERENT psum tiles: a cast (DVE read) of qt's tile would
        # otherwise WAR-serialize the next qt's partB matmuls (tile-level
        # dependency).  qt15 reuses stA/stB third regions -- its matmuls
        # run last, after qt12/qt13's casts have long drained.
        po_map = [stA[:, 0:384], stA[:, QB:QB + 384],          # qt12
                  stB[:, 0:384], stB[:, QB:QB + 384],          # qt13
                  poY[0][:, 0:384], poY[1][:, 0:384],          # qt14
                  stA[:, 2 * QB:2 * QB + 384],
                  stB[:, 2 * QB:2 * QB + 384]]                 # qt15
        for p in range(8):
            qt, nb = 12 + p // 2, p % 2
            po = po_map[p]
            pjAB.append(po)
            nc.tensor.matmul(po, yTA[:, qt * 128:(qt + 1) * 128],
                             wpA[:, nb * 384:(nb + 1) * 384],
                             start=True, stop=False)
        # the (qq3,h2) norm -- the very last -- is finished in 128-wide
        # chunks so each qt's partB pair starts as soon as ITS columns are
        # normalized instead of behind the full 512-wide multiply
        due3, (yt3, bc3, h3, qq3) = pending.pop()
        while pending:
            norm_finish(pending.pop(0)[1])
        # per-qt pipelining: chunked normalize (DVE), row-half dup for odd
        # qt (co-execution pairing), partB pair (PE), casts split DVE/ACT,
        # two half-DMAs; each qt's chain overlaps the next qt's matmuls
        for qx in range(4):
            qt = 12 + qx
            lo = 64 * (qx % 2)
            c0 = qx * 128
            nc.vector.scalar_tensor_tensor(
                out=yTB[0:64, qt * 128:(qt + 1) * 128],
                in0=yt3[0:64, c0:c0 + 128], scalar=1.0,
                in1=bc3[:, c0:c0 + 128], op0=OP.mult, op1=OP.mult)
            if lo:
                nc.vector.tensor_copy(
                    out=yTB[64:128, qt * 128:(qt + 1) * 128],
                    in_=yTB[0:64, qt * 128:(qt + 1) * 128])
            for nb in range(2):
                nc.tensor.matmul(pjAB[qx * 2 + nb],
                                 yTB[lo:lo + 64, qt * 128:(qt + 1) * 128],
                                 wpB[lo:lo + 64, nb * 384:(nb + 1) * 384],
                                 start=False, stop=True)
            ob = ostage.tile([128, C], MMDT, tag="ob", name=f"ob{qt}")
            nc.vector.tensor_copy(out=ob[:, 0:384], in_=pjAB[qx * 2])
            nc.scalar.activation(ob[:, 384:768], pjAB[qx * 2 + 1],
                                 AF.Copy)
            nc.sync.dma_start(out=out[qt * 128:(qt + 1) * 128, 0:384],
                              in_=ob[:, 0:384])
            nc.sync.dma_start(out=out[qt * 128:(qt + 1) * 128, 384:768],
                              in_=ob[:, 384:768])


def _build_program():
    nc = bacc.Bacc("TRN2", target_bir_lowering=False, debug=False,
                   num_devices=NCORES)
    aps = {
        "xT": nc.dram_tensor("xT", [C, N], MMDT, kind="ExternalInput").ap(),
        "wq": nc.dram_tensor("wq", [C, 128], MMDT, kind="ExternalInput").ap(),
        "wk": nc.dram_tensor("wk", [C, 128], MMDT, kind="ExternalInput").ap(),
        "wqk2": nc.dram_tensor("wqk2", [C, 128], MMDT,
                               kind="ExternalInput").ap(),
        "wv": nc.dram_tensor("wv", [C, MYC], MMDT, kind="ExternalInput").ap(),
        "wp": nc.dram_tensor("wp", [MYC, C], MMDT, kind="ExternalInput").ap(),
        "bq": nc.dram_tensor("bq", [128, 1], F32, kind="ExternalInput").ap(),
        "bk": nc.dram_tensor("bk", [128, 1], F32, kind="ExternalInput").ap(),
        "bqk2": nc.dram_tensor("bqk2", [128, 1], F32,
                               kind="ExternalInput").ap(),
        "bv": nc.dram_tensor("bv", [1, MYC], F32, kind="ExternalInput").ap(),
        "out": nc.dram_tensor("out", [N, C], MMDT, kind="ExternalOutput").ap(),
    }
    with tile.TileContext(nc) as tc:
        import contextlib
        with contextlib.ExitStack() as ctx:
            pools = {
                "persist": ctx.enter_context(tc.tile_pool(name="persist", bufs=1)),
                "et": ctx.enter_context(tc.tile_pool(name="et", bufs=LAG + 1)),
                "small": ctx.enter_context(tc.tile_pool(name="small", bufs=3)),
                "ostage": ctx.enter_context(tc.tile_pool(name="ostage", bufs=4)),
            }
            _emit(nc, tc, pools, aps)
    nc.compile()
    return nc


_PROGRAM_CACHE = {}


def _get_program():
    if "nc" not in _PROGRAM_CACHE:
        _PROGRAM_CACHE["nc"] = _build_program()
    return _PROGRAM_CACHE["nc"]


def make_in_maps(x, Wq, bq, Wk, bk, Wv, bv, Wp, bp):
    scale = 1.0 / math.sqrt(DH)
    xTb = [np.ascontiguousarray(x[b].T) for b in range(B)]
    wire = mybir.dt.np(MMDT)
    in_maps = []
    for c in range(NCORES):
        b, hg = c // CPG, c % CPG
        cols = slice(hg * MYC, (hg + 1) * MYC)
        wqc = Wq[:, cols] * np.float32(scale)
        wkc = Wk[:, cols]
        in_maps.append({
            "xT": xTb[b].astype(wire),
            "wq": np.ascontiguousarray(wqc[:, 0:128]).astype(wire),
            "wk": np.ascontiguousarray(wkc[:, 0:128]).astype(wire),
            "wqk2": np.ascontiguousarray(
                np.concatenate([wqc[:, 128:192], wkc[:, 128:192]],
                               axis=1)).astype(wire),
            "wv": np.ascontiguousarray(Wv[:, cols]).astype(wire),
            "wp": np.ascontiguousarray(Wp[cols, :]).astype(wire),
            "bq": (bq[cols][0:128] * np.float32(scale)
                   ).reshape(128, 1).astype(np.float32),
            "bk": bk[cols][0:128].reshape(128, 1).astype(np.float32),
            "bqk2": np.concatenate(
                [bq[cols][128:192] * np.float32(scale), bk[cols][128:192]]
            ).reshape(128, 1).astype(np.float32),
            "bv": bv[cols].reshape(1, MYC).astype(np.float32),
        })
    return in_maps


def assemble(results, bp):
    out = np.empty((B, N, C), np.float32)
    for b in range(B):
        acc = results[b * CPG]["out"].astype(np.float64)
        for c in range(b * CPG + 1, (b + 1) * CPG):
            acc = acc + results[c]["out"]
        out[b] = (acc + bp.astype(np.float64)).astype(np.float32)
    return out


def kernel(x, Wq, bq, Wk, bk, Wv, bv, Wp, bp, **extra_kwargs):
    x = np.asarray(x, np.float32)
    Wq = np.asarray(Wq, np.float32)
    Wk = np.asarray(Wk, np.float32)
    Wv = np.asarray(Wv, np.float32)
    Wp = np.asarray(Wp, np.float32)
    bq = np.asarray(bq, np.float32)
    bk = np.asarray(bk, np.float32)
    bv = np.asarray(bv, np.float32)
    bp = np.asarray(bp, np.float32)

    nc = _get_program()
    in_maps = make_in_maps(x, Wq, bq, Wk, bk, Wv, bv, Wp, bp)
    res = bass_utils.run_bass_kernel_spmd(nc, in_maps,
                                          core_ids=list(range(NCORES)))
    return assemble(res.results, bp)


# revision 35
# speedup vs baseline: 1.0058x; 1.0058x over previous
"""Multi-head attention (B=2, N=2048, C=768, H=12, DH=64) on 8 Trainium2 cores.

Sharding: data-parallel on batch (cores 0-3 -> b=0, cores 4-7 -> b=1),
tensor-parallel on heads within each group (3 heads/core: Wq/Wk/Wv column
slices, Wp row slices).  Each core emits its partial projection output
[N, C]; the host sums the 4 partials per batch and adds bp (cheaper than a
device collective at this size).

Per-core dataflow (feature-major, transpose-free, fp16 operands / fp32 psum):
  - host supplies xT = x[b].T  [C, N] in fp16; h2's q and k weight columns
    are host-packed into one [C,128] tensor so all qk-proj matmuls are M=128
  - PE warmup: dummy matmuls on a memset tile run during the initial DMA
    wait so the HAM clock-gate reaches K=8/8 before real work starts; a
    tiny Exp at t=0 preloads the ACT table set (~2.7us off the stream);
    more dummies bridge the phase-1 -> stream pool barrier (any PE gap
    >~2.5us re-throttles the clock gate for 7-34us)
  - qT,kT [64, N] per head = W.T @ xT; biases via the ACT Identity bias
    port (per-partition column) instead of K=1 matmuls; each head's 64
    dims then duplicated onto both PE-row halves so score matmuls pair
    even/odd kj tiles on disjoint PE row halves (co-execute)
  - h2's psum drains through ACT only (nt-outer, pipelined per-nt), so
    the pool-close barrier that gates the v/score psum pools never waits
    on the DVE queue (which carries the big h0/h1 dup copies)
  - v [N, 192] token-major from xT as lhsT, with a ones column per head;
    bv added by the DVE psum->sbuf copy against a gpsimd-broadcast bias
    tile, not a K=1 matmul; a few v tiles are emitted BEFORE the first
    score groups so the PE stays busy across the transition
  - phase 3 is one continuous stream over (qq, head, kj): 192 score tiles
    STt [kj,qi] grouped 3 per [128,1536] psum tile; ONE exp ACT op per
    group; yT accumulation consumes ET LAG groups behind so transient PE
    detours (proj tiles, normalize) never starve the ACT engine
  - yT_aug[65, qi] = [v_h | 1].T @ ET accumulated over kj; row 64 = denom
  - normalize: denom -> sbuf copy, reciprocal_approx_fast, gpsimd
    partition_broadcast, fused multiply deferred two groups
  - out[qi, C] partial = yT (stationary) @ Wp rows: one single-group
    detour per row-tile (A,A then B,B so each stationary loads once; the
    trailing partB co-executes with the next group's odd-kj score MM);
    the final four tiles pipeline chunked-normalize -> partB pair ->
    casts (split DVE/ACT) -> two half-DMAs per tile
"""

import math

import numpy as np

import concourse.bacc as bacc
import concourse.bass as bass
import concourse.mybir as mybir
import concourse.tile as tile
from concourse import bass_utils

B, N, C, H, DH = 2, 2048, 768, 12, 64
NCORES = 8
CPG = 4                  # cores per batch group
HPC = H // CPG           # heads per core = 3
MYC = HPC * DH           # per-core feature width = 192
KC = C // 128            # contraction chunks = 6
NTT = N // 128           # token tiles = 16
QB = 512                 # qi block (psum bank width, fp32)
LAG = 7                  # ET ring depth: yt consumption trails ACT by LAG
NWARM = 14               # dummy warmup matmuls during the initial DMA wait
                         # (~4.5us: engines unblock ~7.5us after t0, the
                         # first xT chunk lands ~11.7us; more would push the
                         # real matmuls back since the PE queue is FIFO)
F32 = mybir.dt.float32
I16 = mybir.dt.int16
MMDT = mybir.dt.float16  # matmul operand dtype
AF = mybir.ActivationFunctionType
OP = mybir.AluOpType

EXP_SHIFT = -3.0         # exp(s + EXP_SHIFT); cancels between num and denom
# fp16-domain Schraudolph fast exp: bits16(exp(s-3)) ~ round(A16*s + B16).
# Scores are in [-6.1, +6.3] (fixed input seed), so bits stay in
# [~1870, ~20200] -- no int16 overflow, no sign-flip underflow.
F16A = 1024.0 / math.log(2.0)                      # 2^10 * log2(e)
F16B = 15360.0 - 44.7 + EXP_SHIFT * F16A
# groups whose exp runs on DVE (int16 Schraudolph) instead of ACT.
# Scattered offload groups leave a +-3% sawtooth on just THOSE token
# blocks' weights; rows whose attention concentrates there see the full
# error (sim: 1.98e-2, at the gate).  Offload is only safe per whole
# (qq,h) row -- the common-mode error then cancels in the softmax ratio.
DVE_EXP_GROUPS = frozenset()


def _bcast_parts(ap, nparts):
    """Partition-stride-0 broadcast view of a [1, F] AP (DMA source only)."""
    return bass.AP(tensor=ap.tensor, offset=ap.offset,
                   ap=[[0, nparts]] + [list(d) for d in ap.ap[1:]])


def _emit(nc, tc, pools, aps):
    xT, wq, wk, wqk2, wv, wp, bq, bk, bqk2, bv, out = (
        aps["xT"], aps["wq"], aps["wk"], aps["wqk2"], aps["wv"], aps["wp"],
        aps["bq"], aps["bk"], aps["bqk2"], aps["bv"], aps["out"],
    )
    persist = pools["persist"]
    et_pool = pools["et"]
    small = pools["small"]
    ostage = pools["ostage"]

    # ---- persistent SBUF tensors ----
    xT_sb = persist.tile([128, KC * N], MMDT, tag="xT_sb")
    wq_sb = persist.tile([128, KC * 128], MMDT, tag="wq_sb")
    wk_sb = persist.tile([128, KC * 128], MMDT, tag="wk_sb")
    wqk2_sb = persist.tile([128, KC * 128], MMDT, tag="wqk2_sb")
    wv_sb = persist.tile([128, KC * MYC], MMDT, tag="wv_sb")
    wpA = persist.tile([128, C], MMDT, tag="wpA")
    # wpB rows 64:128 duplicate rows 0:64 (same DRAM source, two DMAs) so
    # the final partB matmuls can run on either PE row half
    wpB = persist.tile([128, C], MMDT, tag="wpB")
    bqc = persist.tile([128, 1], F32, tag="bqc")
    bkc = persist.tile([128, 1], F32, tag="bkc")
    bqk2c = persist.tile([128, 1], F32, tag="bqk2c")
    bv_row = persist.tile([1, MYC], F32, tag="bv_row")
    bv_bc = persist.tile([128, MYC], F32, tag="bv_bc")
    shift_col = persist.tile([128, 1], F32, tag="shift_col")
    warm_x = persist.tile([128, QB], MMDT, tag="warm_x")
    tiny = persist.tile([1, 8], F32, tag="tiny")
    tiny2 = persist.tile([1, 8], F32, tag="tiny2")
    # compact projections (h0 on parts 0:64, h1 on 64:128; h2 separate)
    qTA = persist.tile([128, N], MMDT, tag="qTA")
    kTA = persist.tile([128, N], MMDT, tag="kTA")
    # partition-duplicated k/q halves for even/odd kj pair packing:
    # qTDx packs h0's dup (upper half) + h1's dup (lower half); h0-even
    # and h1-odd read qTA/kTA directly; h2 fully duplicated in qTD2
    qTDx = persist.tile([128, N], MMDT, tag="qTDx")
    kTDx = persist.tile([128, N], MMDT, tag="kTDx")
    qTD2 = persist.tile([128, N], MMDT, tag="qTD2")
    kTD2 = persist.tile([128, N], MMDT, tag="kTD2")
    v_sb = persist.tile([128, NTT * HPC * 65], MMDT, tag="v_sb")
    yTA = persist.tile([128, N], MMDT, tag="yTA")
    # yTB rows 0:64 = h2's yT; rows 64:128 = a dup of the last block so the
    # final partB matmuls co-execute pairwise on disjoint PE row halves
    yTB = persist.tile([128, N], MMDT, tag="yTB")

    # warmup operand + ACT exp-table preload: both ready ~instantly so the
    # PE and ACT start before the first input DMA lands
    nc.vector.memset(warm_x, 0.02)
    nc.vector.memset(tiny, 0.0)
    nc.scalar.activation(tiny2, tiny, AF.Exp)

    # ---- input DMAs: qk-proj operands first so phase 1 starts ASAP ----
    # only the three tensors the q/k passes consume go in the hot loop;
    # wqk2 (needed ~29us in) would steal 25% of the early HBM bandwidth
    # and stall the DMA-paced q/k matmul interleave
    for kc in range(KC):
        nc.sync.dma_start(out=xT_sb[:, kc * N:(kc + 1) * N],
                          in_=xT[kc * 128:(kc + 1) * 128, :])
        nc.sync.dma_start(out=wq_sb[:, kc * 128:(kc + 1) * 128],
                          in_=wq[kc * 128:(kc + 1) * 128, :])
        nc.sync.dma_start(out=wk_sb[:, kc * 128:(kc + 1) * 128],
                          in_=wk[kc * 128:(kc + 1) * 128, :])
    nc.sync.dma_start(out=bqc, in_=bq)
    nc.sync.dma_start(out=bkc, in_=bk)
    nc.sync.dma_start(out=bqk2c, in_=bqk2)
    for kc in range(KC):
        nc.sync.dma_start(out=wqk2_sb[:, kc * 128:(kc + 1) * 128],
                          in_=wqk2[kc * 128:(kc + 1) * 128, :])
    for kc in range(KC):
        nc.sync.dma_start(out=wv_sb[:, kc * MYC:(kc + 1) * MYC],
                          in_=wv[kc * 128:(kc + 1) * 128, :])
    nc.sync.dma_start(out=bv_row, in_=bv)
    nc.sync.dma_start(out=wpA, in_=wp[0:128, :])
    nc.sync.dma_start(out=wpB[0:64, :], in_=wp[128:MYC, :])
    nc.sync.dma_start(out=wpB[64:128, :], in_=wp[128:MYC, :])
    nc.gpsimd.partition_broadcast(bv_bc, bv_row)
    nc.vector.memset(shift_col, EXP_SHIFT)
    # pre-fill v_sb with 1.0: the per-head ones-columns (denominator rows
    # of the yt matmuls) then need no per-tile copies; the 64-wide value
    # copies overwrite their slices
    nc.vector.memset(v_sb, 1.0)

    # ---- phase 1: q/k/h2-combined projections (M=128 passes) ----
    with tc.tile_pool(name="ps_qk", bufs=2, space="PSUM") as ps_qk:
        pssQ = [ps_qk.tile([128, QB], F32, tag="ps_qkA", bufs=4,
                           name=f"ps_q{_i}") for _i in range(N // QB)]
        pssK = [ps_qk.tile([128, QB], F32, tag="ps_qkB", bufs=4,
                           name=f"ps_k{_i}") for _i in range(N // QB)]
        # HAM warmup: dummy matmuls on the memset tile keep the PE busy
        # (and the clock gate at 8/8) while the first xT/w DMAs land; the
        # real kc=0 matmul re-opens the bank with start=True
        for i in range(NWARM):
            nc.tensor.matmul(pssQ[0], warm_x[:, 0:128], warm_x,
                             start=(i == 0), stop=(i == NWARM - 1))
        # q and k interleaved per kc chunk so matmul consumption stays
        # behind the xT DMA supply
        for kc in range(KC):
            for nt in range(N // QB):
                nc.tensor.matmul(
                    pssQ[nt], wq_sb[:, kc * 128:(kc + 1) * 128],
                    xT_sb[:, kc * N + nt * QB: kc * N + nt * QB + QB],
                    start=(kc == 0), stop=(kc == KC - 1))
            for nt in range(N // QB):
                nc.tensor.matmul(
                    pssK[nt], wk_sb[:, kc * 128:(kc + 1) * 128],
                    xT_sb[:, kc * N + nt * QB: kc * N + nt * QB + QB],
                    start=(kc == 0), stop=(kc == KC - 1))
        # psum->sbuf move on the (pre-stream idle) ACT engine with the
        # bias folded into the activation's per-partition bias port
        for nt in range(N // QB):
            nc.scalar.activation(qTA[:, nt * QB:(nt + 1) * QB], pssQ[nt],
                                 AF.Identity, bias=bqc)
            nc.scalar.activation(kTA[:, nt * QB:(nt + 1) * QB], pssK[nt],
                                 AF.Identity, bias=bkc)
        # h0/h1 duplicated halves can be built as soon as q/k biases land
        # (DVE; gpsimd's tensor_copy is ~10x slower).  The [64:128] dups
        # feed the very first score groups (h0 odd-kj), so they go first.
        nc.vector.tensor_copy(out=qTDx[64:128, :], in_=qTA[0:64, :])
        nc.vector.tensor_copy(out=kTDx[64:128, :], in_=kTA[0:64, :])
        nc.vector.tensor_copy(out=qTDx[0:64, :], in_=qTA[64:128, :])
        nc.vector.tensor_copy(out=kTDx[0:64, :], in_=kTA[64:128, :])
        # combined h2 pass: psum rows 0:64 = q-h2, rows 64:128 = k-h2
        pss2 = [ps_qk.tile([128, QB], F32, tag="ps_qkA", bufs=4,
                           name=f"ps_2{_i}") for _i in range(N // QB)]
        # nt-OUTER so each pss2 tile finishes early and its drain chain
        # (ACT copy + DVE cast + dups, ~1.3us) pipelines under the next
        # nt's matmuls; kc-outer would serialize the whole ~5us drain
        # after the last h2 matmul, and everything downstream (the v/score
        # psum pools) waits on that drain via the pool-close barrier
        for nt in range(N // QB):
            for kc in range(KC):
                nc.tensor.matmul(
                    pss2[nt], wqk2_sb[:, kc * 128:(kc + 1) * 128],
                    xT_sb[:, kc * N + nt * QB: kc * N + nt * QB + QB],
                    start=(kc == 0), stop=(kc == KC - 1))
            sl = slice(nt * QB, (nt + 1) * QB)
            # BOTH h2 psum copies on ACT (idle pre-stream): then only ACT
            # reads pss2, so the pool-close barrier doesn't wait on the
            # DVE queue (which carries the big h0/h1 dups); the dup chunks
            # (SBUF->SBUF, not pss2 readers) go on DVE and pipeline per-nt
            nc.scalar.activation(kTD2[64:128, sl], pss2[nt][64:128, :],
                                 AF.Identity, bias=bqk2c[64:128, :])
            nc.scalar.activation(qTD2[0:64, sl], pss2[nt][0:64, :],
                                 AF.Identity, bias=bqk2c[0:64, :])
            nc.vector.tensor_copy(out=qTD2[64:128, sl], in_=qTD2[0:64, sl])
            nc.vector.tensor_copy(out=kTD2[0:64, sl], in_=kTD2[64:128, sl])
        # PE filler: the pool-close barrier waits ~2us for the last pss2
        # drain; dummy matmuls (no deps) keep the PE busy so the HAM
        # activity monitor never sees an idle window and re-throttles
        pfill = ps_qk.tile([128, QB], F32, tag="ps_qkA", bufs=4,
                           name="pfill")
        for i in range(14):
            nc.tensor.matmul(pfill, warm_x[:, 0:128], warm_x,
                             start=(i == 0), stop=(i == 13))

    # ---- phases 2+3: v projection + score stream share the PSUM pools ----
    def vh_ap(kj, h):
        base = (kj * HPC + h) * 65
        return v_sb[:, base:base + 65]

    # normalize phase 1: denom row -> sbuf, fast reciprocal, then a
    # gpsimd partition_broadcast (all-SBUF, so legal on Pool).  The fused
    # multiply (phase 2) is DEFERRED two groups so its wait never
    # head-of-line-blocks the DVE queue.
    def norm_start(yt, h, qq):
        den = small.tile([1, QB], F32, tag="den")
        nc.vector.tensor_copy(out=den, in_=yt[64:65, :])
        rec = small.tile([1, QB], F32, tag="rec")
        nc.vector.reciprocal_approx_fast(rec, den)
        bc = small.tile([64, QB], F32, tag="bc_sb")
        nc.gpsimd.partition_broadcast(bc, rec)
        return (yt, bc, h, qq)

    def norm_finish(state):
        yt, bc, h, qq = state
        q0 = qq * QB
        ydst = yTA[0:64, :] if h == 0 else (
            yTA[64:128, :] if h == 1 else yTB[0:64, :])
        nc.vector.scalar_tensor_tensor(
            out=ydst[:, q0:q0 + QB], in0=yt[0:64, :], scalar=1.0, in1=bc,
            op0=OP.mult, op1=OP.mult,
        )

    def proj_full(ps_st, qt):
        # one output row-tile per detour: A,A then B,B (each stationary
        # loaded once), casts, DMA.  A single ~1us detour holds the st
        # slot only ~1.5 groups (vs 2.5 when split across two groups), so
        # the score stream's slot handoff doesn't stall the PE; the
        # trailing partB matmul co-executes with the next group's odd-kj
        # score matmul (disjoint PE row halves)
        stt = ps_st.tile([128, 3 * QB], F32, tag="st", name=f"pj{qt}")
        ob = ostage.tile([128, C], MMDT, tag="ob", name=f"ob{qt}")
        for nb in range(2):
            nc.tensor.matmul(stt[:, nb * QB: nb * QB + 384],
                             yTA[:, qt * 128:(qt + 1) * 128],
                             wpA[:, nb * 384:(nb + 1) * 384],
                             start=True, stop=False)
        for nb in range(2):
            nc.tensor.matmul(stt[:, nb * QB: nb * QB + 384],
                             yTB[0:64, qt * 128:(qt + 1) * 128],
                             wpB[0:64, nb * 384:(nb + 1) * 384],
                             start=False, stop=True)
        # ONE strided cast covering both po regions (block stride 512 in
        # psum -> contiguous 768 in ob): the st slot is freed by its LAST
        # reader, and a single op ends ~0.5us before two serial casts
        # would (mid-stream ACT is busy, so offloading one there would
        # queue behind a pending exp and hold the slot even longer)
        s0 = stt[:, 0:384]
        o0 = ob[:, 0:384]
        nc.vector.tensor_copy(
            out=bass.AP(tensor=o0.tensor, offset=o0.offset,
                        ap=[list(o0.ap[0]), [384, 2], [1, 384]]),
            in_=bass.AP(tensor=s0.tensor, offset=s0.offset,
                        ap=[list(s0.ap[0]), [QB, 2], [1, 384]]))
        nc.sync.dma_start(out=out[qt * 128:(qt + 1) * 128, :], in_=ob)

    stream = [(qq, h, kj)
              for qq in range(4) for h in range(HPC) for kj in range(NTT)]
    NG = len(stream) // 3  # 64 groups of 3 score tiles

    # proj for block qq interleaved into block qq+1's stream, one nb-half
    # per group; slot allocations stay 2 groups apart (parity-preserving);
    # keyed by CONSUMED group
    proj_at = {}
    # even goff: fires at an odd consumed-group, so the next EMITTED
    # group (cg+LAG+1, LAG=7) starts with an odd-kj score matmul on PE
    # rows 64:128 -- the trailing partB matmul (rows 0:64) co-executes
    for qq in range(3):
        for j, goff in enumerate((8, 10, 12, 14)):
            qt = qq * 4 + j
            proj_at.setdefault((qq + 1) * 16 + goff, []).append(qt)

    def st_srcs(h, kj):
        if kj % 2 == 0:      # PE rows 0:64
            kt, qt_ = ((kTA, qTA), (kTDx, qTDx), (kTD2, qTD2))[h]
            lo = 0
        else:                # PE rows 64:128
            kt, qt_ = ((kTDx, qTDx), (kTA, qTA), (kTD2, qTD2))[h]
            lo = 64
        return kt, qt_, lo

    def emit_st_group(ps_st, g):
        entries = [stream[3 * g + j] for j in range(3)]
        st = ps_st.tile([128, 3 * QB], F32, tag="st", name=f"st{g}")
        for j, (qq, h, kj) in enumerate(entries):
            kt, qt_, lo = st_srcs(h, kj)
            nc.tensor.matmul(
                st[:, j * QB:(j + 1) * QB],
                kt[lo:lo + 64, kj * 128:(kj + 1) * 128],
                qt_[lo:lo + 64, qq * QB:(qq + 1) * QB],
                start=True, stop=True,
            )
        if g in DVE_EXP_GROUPS:
            # fp16-domain Schraudolph: one DVE op producing the fp16 BIT
            # pattern as int16; the ring stores the tile + a bitcast flag
            eti = et_pool.tile([128, 3 * QB], I16, tag="et", name=f"et{g}")
            nc.vector.tensor_scalar(out=eti, in0=st, scalar1=F16A,
                                    scalar2=F16B, op0=OP.mult, op1=OP.add)
            return (eti, True, entries)
        et = et_pool.tile([128, 3 * QB], MMDT, tag="et", name=f"et{g}")
        nc.scalar.activation(et, st, AF.Exp, bias=shift_col[:, :])
        return (et, False, entries)

    with tc.tile_pool(name="ps_st", bufs=2, space="PSUM") as ps_st, \
         tc.tile_pool(name="ps_yt", bufs=2, space="PSUM") as ps_yt:
        ring = []

        def emit_v(nt):
            ps = ps_yt.tile([128, MYC], F32, tag="yt", name=f"ps_v{nt}")
            for kc in range(KC):
                nc.tensor.matmul(
                    ps,
                    xT_sb[:, kc * N + nt * 128: kc * N + nt * 128 + 128],
                    wv_sb[:, kc * MYC:(kc + 1) * MYC],
                    start=(kc == 0), stop=(kc == KC - 1),
                )
            for h in range(HPC):
                base = (nt * HPC + h) * 65
                nc.vector.tensor_tensor(
                    out=v_sb[:, base:base + 64],
                    in0=ps[:, h * 64:(h + 1) * 64],
                    in1=bv_bc[:, h * 64:(h + 1) * 64], op=OP.add)

        # v tiles FIRST: they depend only on long-resident xT/wv and their
        # psum pool's banks drained early, so the PE stays busy during the
        # ~2us h2-psum drain that gates the first score groups (a PE gap
        # here costs a HAM re-throttle, worth tens of us)
        for nt in range(4):
            emit_v(nt)
        ring.append(emit_st_group(ps_st, 0))
        ring.append(emit_st_group(ps_st, 1))
        for k in range(2, LAG):
            s = 4 + (NTT - 4) * (k - 2) // (LAG - 2)
            e = 4 + (NTT - 4) * (k - 1) // (LAG - 2)
            for nt in range(s, e):
                emit_v(nt)
            ring.append(emit_st_group(ps_st, k))

        # ---- phase 3 main loop ----
        yt_cur = {}
        pending = []   # (due consumed-group, norm state)
        pjAB = []
        consumed = 0
        gi = LAG
        while consumed < NG:
            if gi < NG:
                ring.append(emit_st_group(ps_st, gi))
            gi += 1
            ncons = min(1 if gi <= NG else 2, len(ring))
            batch, ring[:] = ring[:ncons], ring[ncons:]
            for pet, is_i16, pentries in batch:
                cg = consumed
                consumed += 1
                for j, (qq, h, kj) in enumerate(pentries):
                    if kj == 0:
                        yt_cur[(qq, h)] = ps_yt.tile(
                            [65, QB], F32, tag="yt", name=f"yt{qq}_{h}")
                    src = pet[:, j * QB:(j + 1) * QB]
                    if is_i16:
                        src = src.bitcast(MMDT)
                    nc.tensor.matmul(yt_cur[(qq, h)], vh_ap(kj, h), src,
                                     start=(kj == 0), stop=(kj == NTT - 1))
                    if kj == NTT - 1:
                        pending.append(
                            (cg + 2, norm_start(yt_cur.pop((qq, h)), h, qq)))
                while pending and pending[0][0] <= cg:
                    norm_finish(pending.pop(0)[1])
                for qt in proj_at.get(cg + 1, []):
                    proj_full(ps_st, qt)
        # final block's h0+h1 proj contribution emitted after every other
        # st-pool user (the partA tiles hold both slots until partB):
        # 8 bank-aligned po regions (3+3 in the st slots, 2 from yt pool)
        stA = ps_st.tile([128, 3 * QB], F32, tag="st", name="pjtA")
        stB = ps_st.tile([128, 3 * QB], F32, tag="st", name="pjtB")
        poY = [ps_yt.tile([128, 384], F32, tag="yt", name=f"poY{_i}")
               for _i in range(2)]
        # po regions are spread so consecutive qt's regions live on
        # DIFFERENT psum tiles: a cast (DVE read) of qt's tile would
        # otherwise WAR-serialize the next qt's partB matmuls (tile-level
        # dependency).  qt15 reuses stA/stB third regions -- its matmuls
        # run last, after qt12/qt13's casts have long drained.
        po_map = [stA[:, 0:384], stA[:, QB:QB + 384],          # qt12
                  stB[:, 0:384], stB[:, QB:QB + 384],          # qt13
                  poY[0][:, 0:384], poY[1][:, 0:384],          # qt14
                  stA[:, 2 * QB:2 * QB + 384],
                  stB[:, 2 * QB:2 * QB + 384]]                 # qt15
        for p in range(8):
            qt, nb = 12 + p // 2, p % 2
            po = po_map[p]
            pjAB.append(po)
            nc.tensor.matmul(po, yTA[:, qt * 128:(qt + 1) * 128],
                             wpA[:, nb * 384:(nb + 1) * 384],
                             start=True, stop=False)
        # the (qq3,h2) norm -- the very last -- is finished in 128-wide
        # chunks so each qt's partB pair starts as soon as ITS columns are
        # normalized instead of behind the full 512-wide multiply
        due3, (yt3, bc3, h3, qq3) = pending.pop()
        while pending:
            norm_finish(pending.pop(0)[1])
        # per-qt pipelining: chunked normalize (DVE), row-half dup for odd
        # qt (co-execution pairing), partB pair (PE), casts split DVE/ACT,
        # two half-DMAs; each qt's chain overlaps the next qt's matmuls
        for qx in range(4):
            qt = 12 + qx
            lo = 64 * (qx % 2)
            c0 = qx * 128
            nc.vector.scalar_tensor_tensor(
                out=yTB[0:64, qt * 128:(qt + 1) * 128],
                in0=yt3[0:64, c0:c0 + 128], scalar=1.0,
                in1=bc3[:, c0:c0 + 128], op0=OP.mult, op1=OP.mult)
            if lo:
                nc.vector.tensor_copy(
                    out=yTB[64:128, qt * 128:(qt + 1) * 128],
                    in_=yTB[0:64, qt * 128:(qt + 1) * 128])
            for nb in range(2):
                nc.tensor.matmul(pjAB[qx * 2 + nb],
                                 yTB[lo:lo + 64, qt * 128:(qt + 1) * 128],
                                 wpB[lo:lo + 64, nb * 384:(nb + 1) * 384],
                                 start=False, stop=True)
            ob = ostage.tile([128, C], MMDT, tag="ob", name=f"ob{qt}")
            nc.vector.tensor_copy(out=ob[:, 0:384], in_=pjAB[qx * 2])
            nc.scalar.activation(ob[:, 384:768], pjAB[qx * 2 + 1],
                                 AF.Copy)
            nc.sync.dma_start(out=out[qt * 128:(qt + 1) * 128, 0:384],
                              in_=ob[:, 0:384])
            nc.sync.dma_start(out=out[qt * 128:(qt + 1) * 128, 384:768],
                              in_=ob[:, 384:768])


def _build_program():
    nc = bacc.Bacc("TRN2", target_bir_lowering=False, debug=False,
                   num_devices=NCORES)
    aps = {
        "xT": nc.dram_tensor("xT", [C, N], MMDT, kind="ExternalInput").ap(),
        "wq": nc.dram_tensor("wq", [C, 128], MMDT, kind="ExternalInput").ap(),
        "wk": nc.dram_tensor("wk", [C, 128], MMDT, kind="ExternalInput").ap(),
        "wqk2": nc.dram_tensor("wqk2", [C, 128], MMDT,
                               kind="ExternalInput").ap(),
        "wv": nc.dram_tensor("wv", [C, MYC], MMDT, kind="ExternalInput").ap(),
        "wp": nc.dram_tensor("wp", [MYC, C], MMDT, kind="ExternalInput").ap(),
        "bq": nc.dram_tensor("bq", [128, 1], F32, kind="ExternalInput").ap(),
        "bk": nc.dram_tensor("bk", [128, 1], F32, kind="ExternalInput").ap(),
        "bqk2": nc.dram_tensor("bqk2", [128, 1], F32,
                               kind="ExternalInput").ap(),
        "bv": nc.dram_tensor("bv", [1, MYC], F32, kind="ExternalInput").ap(),
        "out": nc.dram_tensor("out", [N, C], MMDT, kind="ExternalOutput").ap(),
    }
    with tile.TileContext(nc) as tc:
        import contextlib
        with contextlib.ExitStack() as ctx:
            pools = {
                "persist": ctx.enter_context(tc.tile_pool(name="persist", bufs=1)),
                "et": ctx.enter_context(tc.tile_pool(name="et", bufs=LAG + 1)),
                "small": ctx.enter_context(tc.tile_pool(name="small", bufs=3)),
                "ostage": ctx.enter_context(tc.tile_pool(name="ostage", bufs=4)),
            }
            _emit(nc, tc, pools, aps)
    nc.compile()
    return nc


_PROGRAM_CACHE = {}


def _get_program():
    if "nc" not in _PROGRAM_CACHE:
        _PROGRAM_CACHE["nc"] = _build_program()
    return _PROGRAM_CACHE["nc"]


def make_in_maps(x, Wq, bq, Wk, bk, Wv, bv, Wp, bp):
    scale = 1.0 / math.sqrt(DH)
    xTb = [np.ascontiguousarray(x[b].T) for b in range(B)]
    wire = mybir.dt.np(MMDT)
    in_maps = []
    for c in range(NCORES):
        b, hg = c // CPG, c % CPG
        cols = slice(hg * MYC, (hg + 1) * MYC)
        wqc = Wq[:, cols] * np.float32(scale)
        wkc = Wk[:, cols]
        in_maps.append({
            "xT": xTb[b].astype(wire),
            "wq": np.ascontiguousarray(wqc[:, 0:128]).astype(wire),
            "wk": np.ascontiguousarray(wkc[:, 0:128]).astype(wire),
            "wqk2": np.ascontiguousarray(
                np.concatenate([wqc[:, 128:192], wkc[:, 128:192]],
                               axis=1)).astype(wire),
            "wv": np.ascontiguousarray(Wv[:, cols]).astype(wire),
            "wp": np.ascontiguousarray(Wp[cols, :]).astype(wire),
            "bq": (bq[cols][0:128] * np.float32(scale)
                   ).reshape(128, 1).astype(np.float32),
            "bk": bk[cols][0:128].reshape(128, 1).astype(np.float32),
            "bqk2": np.concatenate(
                [bq[cols][128:192] * np.float32(scale), bk[cols][128:192]]
            ).reshape(128, 1).astype(np.float32),
            "bv": bv[cols].reshape(1, MYC).astype(np.float32),
        })
    return in_maps


def assemble(results, bp):
    out = np.empty((B, N, C), np.float32)
    for b in range(B):
        acc = results[b * CPG]["out"].astype(np.float64)
        for c in range(b * CPG + 1, (b + 1) * CPG):
            acc = acc + results[c]["out"]
        out[b] = (acc + bp.astype(np.float64)).astype(np.float32)
    return out


def kernel(x, Wq, bq, Wk, bk, Wv, bv, Wp, bp, **extra_kwargs):
    x = np.asarray(x, np.float32)
    Wq = np.asarray(Wq, np.float32)
    Wk = np.asarray(Wk, np.float32)
    Wv = np.asarray(Wv, np.float32)
    Wp = np.asarray(Wp, np.float32)
    bq = np.asarray(bq, np.float32)
    bk = np.asarray(bk, np.float32)
    bv = np.asarray(bv, np.float32)
    bp = np.asarray(bp, np.float32)

    nc = _get_program()
    in_maps = make_in_maps(x, Wq, bq, Wk, bk, Wv, bv, Wp, bp)
    res = bass_utils.run_bass_kernel_spmd(nc, in_maps,
                                          core_ids=list(range(NCORES)))
    return assemble(res.results, bp)
